# revision 40
# baseline (speedup 1.0000x reference)
"""Trainium2 Bass kernel for nn_Attention_12266426598027.

GQA attention layer (B=4, S=2048, H=896, 14 q-heads / 2 kv-heads, HD=64,
RoPE theta=1e6, causal) distributed over 8 NeuronCores.

Sharding: core = (batch b, kv-group g) with b in 0..3, g in 0..1. Each core
computes 7 q-heads against its kv head for one batch, including its slice of
the QKV projection and a partial o_proj (448 of the 896 contraction dims).
The two partial o_proj outputs per batch are summed on the host (the
"all-reduce after o_proj" of the tensor-parallel split).

Device schedule (v2 — paired scores + batched exp):
- All matmul operands are bf16 (error budget 2e-2 is ~50x above bf16 matmul
  noise). This enables FWL weight loads and full-rate matmuls.
- Scores use PE row tiling: the RoPE'd q tiles qr[m] hold head pairs
  (2m, 2m+1) on partition halves, and k2 = [k; k] holds the kv head on both
  halves. Two K=64 matmuls at tile rows 0 / 64 run CONCURRENTLY on the two
  array halves (measured 1.85x vs one full-array MM), computing both heads'
  score chunks in ~one matmul slot into the two banks of one [128, 1024]
  PSUM tile. Head 6 pairs its own chunks c / c+1 the same way via
  qd6 = [q6; q6].
- exp runs as ONE 1024-wide ACTIVATE spanning both PSUM banks (measured
  1.27x vs two 512-wide), into a [128, 1024] bf16 probs tile. Diagonal
  chunks with trimmed N are exp'd as two narrower ACTIVATEs.
- Waves = head pairs: (0,1), (2,3), (4,5), then head 6 chunk-paired. Per
  chunk the wave is software-pipelined: scores(c) are emitted BEFORE PV(c-1)
  so the exp latency is hidden behind the next chunk's score matmuls.
- PSUM: sc2 [128,1024] x bufs=2 (4 banks) + pv0/pv1 (2) + proj x bufs=2 (2).
- QKV(j+2) + o_proj chains run as fillers in the PE slack of the ACT-bound
  attention stream (generator queue, one matmul per pop). Row sums come from
  an appended ones-column on V (PV row 64); normalization is one DVE copy to
  free the pv bank, with reciprocal + gpsimd broadcast + multiply deferred
  to the next block.
- RoPE for block j+1 is emitted between waves 1 and 2 of block j; early
  blocks build the rotate-half swap and the k2/qd6 partition-duplicates
  with permutation matmuls + ACT copies, late blocks use sync SBUF-SBUF
  DMAs hidden under attention.
"""
import os
import sys

for _p in ('/opt/trn_rl_repo', '/root/.axon_site'):
    if _p not in sys.path:
        sys.path.insert(0, _p)

import numpy as np

B, S, H = 4, 2048, 896
NH, NKV, HD = 14, 2, 64
NHC, DQ = 7, 448          # q-heads per core, their stacked dim
ROPE_THETA = 1e6
M_SIZES = [128, 128, 128, 128, 64]   # qkv m-tiles over 576 = 448q + 64k + 64v
M_OFFS = [0, 128, 256, 384, 512]
NJ = 4                                # 512-wide q blocks

_PROGRAM_CACHE = {}


def _build_program():
    import concourse.bass as bass
    from concourse import bacc
    import concourse.mybir as mybir
    import concourse.tile as tile
    F32 = mybir.dt.float32
    BF16 = mybir.dt.bfloat16
    ALU = mybir.AluOpType
    AF = mybir.ActivationFunctionType

    nc = bacc.Bacc("TRN2", target_bir_lowering=False, debug=False,
                   num_devices=8)

    xT_d = nc.dram_tensor("xT", [H, S], BF16, kind="ExternalInput").ap()
    wT_d = nc.dram_tensor("wT", [H, 576], BF16, kind="ExternalInput").ap()
    bias_d = nc.dram_tensor("bias", [640], F32, kind="ExternalInput").ap()
    woT_d = nc.dram_tensor("woT", [DQ, H], BF16, kind="ExternalInput").ap()
    cos2_d = nc.dram_tensor("cos2", [128, S], BF16, kind="ExternalInput").ap()
    sinm2_d = nc.dram_tensor("sinm2", [128, S], BF16,
                             kind="ExternalInput").ap()
    ident_d = nc.dram_tensor("ident64", [64, 64], BF16,
                             kind="ExternalInput").ap()
    pswap_d = nc.dram_tensor("pswap", [128, 128], BF16,
                             kind="ExternalInput").ap()
    dup2_d = nc.dram_tensor("dup2", [128, 128], BF16,
                            kind="ExternalInput").ap()
    yT_d = nc.dram_tensor("yT", [H, S], F32, kind="ExternalOutput").ap()

    DEBUG = os.environ.get("KERNEL_DEBUG_OUTPUTS", "0") == "1"
    if DEBUG:
        dbg = {}
        for nm, shp in [("dqkv", [5 * 128, S]), ("dqr", [4 * 128, S]),
                        ("dk2", [128, S]),
                        ("dv", [16 * 128, 128]), ("dattn", [4 * 128, S])]:
            dbg[nm] = nc.dram_tensor(nm, shp, BF16, kind="ExternalOutput").ap()

    with tile.TileContext(nc) as tc:
        with tc.tile_pool(name="persist", bufs=1) as pp, \
             tc.tile_pool(name="small", bufs=1) as psm, \
             tc.tile_pool(name="ps", bufs=1, space="PSUM") as ps, \
             tc.tile_pool(name="sb", bufs=1) as sb:

            xt = [pp.tile([128, S], BF16, tag=f"x{i}", name=f"x{i}")
                  for i in range(7)]
            wt = [pp.tile([128, 576], BF16, tag=f"w{i}", name=f"w{i}")
                  for i in range(7)]
            wo = [pp.tile([128, H], BF16, tag=f"wo{i}", name=f"wo{i}")
                  for i in range(4)]
            qkv = [pp.tile([128, S], BF16, tag=f"qkv{m}", name=f"qkv{m}")
                   for m in range(5)]
            qr = [pp.tile([128, S], BF16, tag=f"qr{m}", name=f"qr{m}")
                  for m in range(4)]
            k2 = pp.tile([128, S], BF16, tag="k2", name="k2")
            # qd6 = [q6; q6]: head 6 on both partition halves so its scores
            # can chunk-pair as two concurrent K=64 row-tile matmuls.
            qd6 = pp.tile([128, S], BF16, tag="qd6", name="qd6")
            # v columns 65:128 are zero-padding (full-width stationary)
            v_sb = [pp.tile([128, 128], BF16, tag=f"v{i}", name=f"v{i}")
                    for i in range(16)]
            attn_all = [pp.tile([128, S], BF16, tag=f"attn{i}",
                                name=f"attn{i}") for i in range(4)]
            cos2t = pp.tile([128, S], BF16, tag="cos2t", name="cos2t")
            sinm2t = pp.tile([128, S], BF16, tag="sinm2t", name="sinm2t")
            warm_sb = pp.tile([128, 512], BF16, tag="warm", name="warm")

            biast = psm.tile([128, 5], F32, name="biast")
            ident = psm.tile([64, 64], BF16, name="ident")
            # pswap[d, m] = 1 iff d == (m flipped within 32-halves of its 64)
            pswap = psm.tile([128, 128], BF16, name="pswap")
            # dup2[p, m] = 1 iff p % 64 == m % 64 (row-duplication stationary)
            dup2 = psm.tile([128, 128], BF16, name="dup2")
            # tri01[k, q] = 1 where q >= k else 0 (diagonal probs mask)
            tri01 = psm.tile([128, 128], BF16, name="tri01")

            # ---- PE warmup: keep HAM busy while setup DMAs land ----------
            nc.vector.memset(warm_sb[:], 0.0)
            # 24 warm MMs span the whole ~9us input-DMA window (the QKV
            # chains are DMA-gated anyway), so HAM reaches 8/8 before the
            # dense projection phase instead of ~11-14us in
            for wmi in range(24):
                # alternate halves so every sc2 psum bank gets initialized
                hf = 512 * (wmi % 2)
                pw = ps.tile([128, 1024], F32, tag="sc2", bufs=2,
                             name=f"warm{wmi}")
                nc.tensor.matmul(pw[:, hf:hf + 512], warm_sb[:, 0:128],
                                 warm_sb[:], start=True, stop=True)

            def dma_x(j):
                cl = slice(512 * j, 512 * j + 512)
                for i in range(7):
                    nc.sync.dma_start(xt[i][:, cl],
                                      xT_d[128 * i:128 * i + 128, cl])

            # ---- setup DMAs, spread across three engine queues so the
            # input load isn't serialized on the sync queue: weights on
            # gpsimd, x on sync, constants + wo on scalar -----------------
            for i in range(7):
                nc.sync.dma_start(wt[i][:], wT_d[128 * i:128 * i + 128, :])
                nc.sync.dma_start(xt[i][:, 0:512], xT_d[128 * i:128 * i + 128,
                                                        0:512])
            nc.sync.dma_start(biast[:], bias_d.rearrange("(m p) -> p m",
                                                         p=128))
            nc.sync.dma_start(ident[:], ident_d[:])
            nc.sync.dma_start(pswap[:], pswap_d[:])
            nc.sync.dma_start(dup2[:], dup2_d[:])
            nc.sync.dma_start(cos2t[:], cos2_d[:])
            nc.sync.dma_start(sinm2t[:], sinm2_d[:])
            for cc in range(4):
                K = 128 if cc < 3 else 64
                nc.sync.dma_start(wo[cc][0:K, :],
                                  woT_d[128 * cc:128 * cc + K, :])
            # duplicate wo's head-6 rows on the upper array half so o_proj
            # cc=3 matmuls can pair as concurrent K=64 row tiles
            nc.sync.dma_start(wo[3][64:128, :], woT_d[384:448, :])
            dma_x(1)
            # v_sb init on gpsimd (idle at start) — on DVE these 32 memsets
            # queue ahead of the QKV bias-adds that gate RoPE(0)
            for i in range(16):
                nc.gpsimd.memset(v_sb[i][:, 64:65], 1.0)
                nc.gpsimd.memset(v_sb[i][:, 65:128], 0.0)
            nc.gpsimd.memset(tri01[:], 1.0)
            nc.gpsimd.affine_select(
                out=tri01[:], in_=tri01[:], compare_op=ALU.is_ge,
                fill=0.0, base=0, pattern=[[1, 128]], channel_multiplier=-1)

            # ---- emitters ------------------------------------------------
            def emit_qkv_chain(j, m):
                """Generator: QKV projection chain for m-tile of block j."""
                M, mo = M_SIZES[m], M_OFFS[m]
                cl = slice(512 * j, 512 * j + 512)
                pst = ps.tile([128, 512], F32, tag="proj", bufs=2,
                              name=f"qkvps{j}_{m}")
                for h in range(7):
                    nc.tensor.matmul(pst[0:M, :], wt[h][:, mo:mo + M],
                                     xt[h][:, cl],
                                     start=(h == 0), stop=(h == 6))
                    yield
                nc.vector.tensor_scalar_add(qkv[m][0:M, cl], pst[0:M, :],
                                            biast[0:M, m:m + 1])
                yield

            def emit_oproj_unit(jb, ot, tail=False):
                """Generator: o_proj chain for output tile ot of block jb.
                tail: sc2 slots are free, so alternate psum tags for a
                4-deep unit pipeline, and split the copy-out + DMA so the
                final output writes start earlier."""
                cl = slice(512 * jb, 512 * jb + 512)
                pst = ps.tile([128, 512], F32, tag="proj", bufs=2,
                              name=f"ops{jb}_{ot}")
                for cc in range(4):
                    K = 128 if cc < 3 else 64
                    nc.tensor.matmul(pst[:],
                                     wo[cc][0:K, 128 * ot:128 * ot + 128],
                                     attn_all[cc][0:K, cl],
                                     start=(cc == 0), stop=(cc == 3))
                    yield
                osb = sb.tile([128, 512], F32, tag="osb", bufs=2,
                              name=f"osb{jb}_{ot}")
                if tail:
                    # ACT is idle in the tail; DVE runs the fin chains.
                    # Halved copy+DMA pipelines the final output writes.
                    cl0 = slice(512 * jb, 512 * jb + 256)
                    cl1 = slice(512 * jb + 256, 512 * jb + 512)
                    nc.scalar.copy(osb[:, 0:256], pst[:, 0:256])
                    nc.sync.dma_start(yT_d[128 * ot:128 * ot + 128, cl0],
                                      osb[:, 0:256])
                    nc.scalar.copy(osb[:, 256:512], pst[:, 256:512])
                    nc.sync.dma_start(yT_d[128 * ot:128 * ot + 128, cl1],
                                      osb[:, 256:512])
                else:
                    nc.vector.tensor_copy(osb[:], pst[:])
                    nc.sync.dma_start(yT_d[128 * ot:128 * ot + 128, cl],
                                      osb[:])
                yield

            def emit_oproj_pair(jb, ot0, tail=False):
                """Generator: o_proj for output tiles (ot0, ot0+1). The two
                K=64 cc=3 matmuls are emitted adjacent on opposite array
                halves (rows 0 / 64) so they run concurrently."""
                ot1 = ot0 + 1
                cl = slice(512 * jb, 512 * jb + 512)
                pA = ps.tile([128, 512], F32, tag="proj", bufs=2,
                             name=f"opA{jb}_{ot0}")
                pB = ps.tile([128, 512], F32, tag="proj", bufs=2,
                             name=f"opB{jb}_{ot1}")
                for cc in range(3):
                    nc.tensor.matmul(pA[:],
                                     wo[cc][:, 128 * ot0:128 * ot0 + 128],
                                     attn_all[cc][:, cl],
                                     start=(cc == 0), stop=False)
                    yield
                for cc in range(3):
                    nc.tensor.matmul(pB[:],
                                     wo[cc][:, 128 * ot1:128 * ot1 + 128],
                                     attn_all[cc][:, cl],
                                     start=(cc == 0), stop=False)
                    yield
                nc.tensor.matmul(pA[:], wo[3][0:64,
                                              128 * ot0:128 * ot0 + 128],
                                 attn_all[3][0:64, cl],
                                 start=False, stop=True)
                nc.tensor.matmul(pB[:], wo[3][64:128,
                                              128 * ot1:128 * ot1 + 128],
                                 attn_all[3][64:128, cl],
                                 start=False, stop=True)
                yield
                for ot, pst in ((ot0, pA), (ot1, pB)):
                    osb = sb.tile([128, 512], F32, tag="osb", bufs=2,
                                  name=f"osb{jb}_{ot}")
                    if tail:
                        for pi in range(2):
                            cs0 = slice(256 * pi, 256 * pi + 256)
                            cld = slice(512 * jb + 256 * pi,
                                        512 * jb + 256 * pi + 256)
                            nc.vector.tensor_copy(osb[:, cs0], pst[:, cs0])
                            nc.sync.dma_start(
                                yT_d[128 * ot:128 * ot + 128, cld],
                                osb[:, cs0])
                    else:
                        nc.vector.tensor_copy(osb[:], pst[:])
                        nc.sync.dma_start(yT_d[128 * ot:128 * ot + 128, cl],
                                          osb[:])
                    yield

            def emit_rope(j, via_pe):
                """RoPE block j (m order 3,0,1,2 so k2/qd6 land first).
                via_pe: build the rotate-half swap and the k2/qd6
                partition-duplicates with permutation matmuls (early blocks
                where the serial sync-DMA queue is exposed); late blocks use
                sync SBUF-SBUF copies, hidden under attention."""
                cl = slice(512 * j, 512 * j + 512)
                for m in (3, 0, 1, 2):
                    tsin = sb.tile([128, 512], BF16, tag="tsin", bufs=2,
                                   name=f"tsin{j}_{m}")
                    if via_pe:
                        xswp = ps.tile([128, 1024], F32, tag="sc2", bufs=2,
                                       name=f"xswp{j}_{m}")
                        nc.tensor.matmul(xswp[:, 0:512], pswap[:],
                                         qkv[m][:, cl], start=True, stop=True)
                        nc.vector.tensor_tensor(tsin[:], xswp[:, 0:512],
                                                sinm2t[:, cl], ALU.mult)
                    else:
                        xsw = sb.tile([128, 512], BF16, tag="xsw", bufs=2,
                                      name=f"xsw{j}_{m}")
                        nc.sync.dma_start(xsw[0:32, :], qkv[m][32:64, cl])
                        nc.sync.dma_start(xsw[32:64, :], qkv[m][0:32, cl])
                        nc.sync.dma_start(xsw[64:96, :], qkv[m][96:128, cl])
                        nc.sync.dma_start(xsw[96:128, :], qkv[m][64:96, cl])
                        nc.vector.tensor_tensor(tsin[:], xsw[:],
                                                sinm2t[:, cl], ALU.mult)
                    nc.vector.tensor_tensor(qr[m][:, cl], qkv[m][:, cl],
                                            cos2t[:, cl], ALU.mult)
                    nc.vector.tensor_tensor(qr[m][:, cl], qr[m][:, cl],
                                            tsin[:], ALU.add)
                    if m == 3:
                        # k2 = dup(qr3[64:128]); qd6 = dup(qr3[0:64])
                        for off, dst in ((64, k2), (0, qd6)):
                            src = qr[3][off:off + 64, cl]
                            if via_pe:
                                dp = ps.tile([128, 1024], F32, tag="sc2",
                                             bufs=2, name=f"dup{j}_{off}")
                                nc.tensor.matmul(dp[:, 0:512],
                                                 dup2[off:off + 64, :], src,
                                                 start=True, stop=True)
                                nc.scalar.copy(dst[:, cl], dp[:, 0:512])
                            else:
                                nc.sync.dma_start(dst[0:64, cl], src)
                                nc.sync.dma_start(dst[64:128, cl], src)

            def emit_vtrans(j):
                for i in range(4 * j, 4 * j + 4):
                    pst = ps.tile([128, 64], BF16, tag="proj", bufs=2,
                                  name=f"vtr{i}")
                    nc.tensor.transpose(
                        pst[:], qkv[4][0:64, 128 * i:128 * i + 128], ident[:])
                    nc.vector.tensor_copy(v_sb[i][:, 0:64], pst[:])

            norm_pend = []

            def norm_stage(j, h, pv_t):
                """One DVE copy frees the pv PSUM bank; the rest of the
                normalization is deferred (attn_all is only read by o_proj
                one iteration later)."""
                stage = sb.tile([65, 512], F32, tag="stage", bufs=7,
                                name=f"st{j}_{h}")
                nc.vector.tensor_copy(stage[:], pv_t[0:65, :])
                norm_pend.append((j, h, stage))

            def norm_fin():
                for j, h, stage in norm_pend:
                    cl = slice(512 * j, 512 * j + 512)
                    # the partition-0 copy is load-bearing: custom-DVE
                    # reciprocal mis-lowers a partition-shifted input AP
                    rsum = sb.tile([1, 512], F32, tag="rsum", bufs=4,
                                   name=f"rs{j}_{h}")
                    nc.vector.tensor_copy(rsum[:], stage[64:65, :])
                    rcp = sb.tile([1, 512], F32, tag="rcp", bufs=4,
                                  name=f"rc{j}_{h}")
                    nc.vector.reciprocal_approx_fast(out=rcp[:],
                                                     in_=rsum[:])
                    rb = sb.tile([64, 512], F32, tag="rb", bufs=4,
                                 name=f"rb{j}_{h}")
                    nc.gpsimd.partition_broadcast(rb[:], rcp[:])
                    dst = attn_all[h // 2][64 * (h % 2):64 * (h % 2) + 64,
                                           cl]
                    nc.vector.tensor_tensor(dst, stage[0:64, :], rb[:],
                                            ALU.mult)
                    if h == 6:
                        nc.vector.tensor_copy(attn_all[3][64:128, cl],
                                              attn_all[3][0:64, cl])
                norm_pend.clear()

            # ---- filler machinery ---------------------------------------
            filler_q = []
            filler_reserve = [0]   # generators held back for the drain

            def pop_filler(n=1):
                for _ in range(n):
                    while len(filler_q) > filler_reserve[0]:
                        try:
                            next(filler_q[0])
                            return
                        except StopIteration:
                            filler_q.pop(0)

            def drain_fillers():
                filler_reserve[0] = 0
                while filler_q:
                    try:
                        next(filler_q[0])
                    except StopIteration:
                        filler_q.pop(0)

            ka_ctr = [0]

            def emit_dummy_unit():
                """Generator: 4 no-reader matmuls into a proj-tag bank.
                Pure PE activity for the ACT-bound last block: keeps the HAM
                window fed so the tail o_proj never drops to half clock."""
                ka_ctr[0] += 1
                pst = ps.tile([128, 512], F32, tag="proj", bufs=2,
                              name=f"dum{ka_ctr[0]}")
                for _ in range(4):
                    nc.tensor.matmul(pst[:], warm_sb[:, 0:128], warm_sb[:],
                                     start=True, stop=True)
                    yield

            def keepalive():
                """One dummy N=512 matmul into the (free) pv1 bank: keeps
                the HAM activity window fed through ACT-bound stretches so
                the tail never drops to half clock. Only legal where pv1 has
                no live accumulation (h6 waves, tail)."""
                ka_ctr[0] += 1
                kps = ps.tile([128, 512], F32, tag="pv1", bufs=1,
                              name=f"ka{ka_ctr[0]}")
                nc.tensor.matmul(kps[:], warm_sb[:, 0:128], warm_sb[:],
                                 start=True, stop=True)

            def drain_gens(gens):
                """Run the filler queue until the given generators (near the
                queue front) are exhausted — a targeted drain that leaves
                later fillers queued."""
                gs = set(map(id, gens))
                while filler_q and any(id(g) in gs for g in filler_q):
                    try:
                        next(filler_q[0])
                    except StopIteration:
                        filler_q.pop(0)

            # ---- attention wave emitters --------------------------------
            def emit_pair_wave(j, m, nkc):
                """Heads (2m, 2m+1): paired K=64 score MMs into the two
                banks of one sc2 tile, one 1024-wide exp, PV pipelined one
                chunk behind the scores."""
                jb = 512 * j
                pv0 = ps.tile([128, 512], F32, tag="pv0", bufs=1,
                              name=f"pv{j}_{2 * m}")
                pv1 = ps.tile([128, 512], F32, tag="pv1", bufs=1,
                              name=f"pv{j}_{2 * m + 1}")
                pend = []   # (c, t, lo, N, probs)

                def flush_pv():
                    while pend:
                        c, t, off, N, pt = pend.pop(0)
                        lo = 512 - N
                        if t >= 0:
                            nc.vector.tensor_tensor(
                                pt[:, off:off + 128], pt[:, off:off + 128],
                                tri01[:], ALU.mult)
                            nc.vector.tensor_tensor(
                                pt[:, 512:640], pt[:, 512:640], tri01[:],
                                ALU.mult)
                        nc.tensor.matmul(pv0[:, lo:512], v_sb[c][:],
                                         pt[:, off:512],
                                         start=(c == 0), stop=(c == nkc - 1))
                        nc.tensor.matmul(pv1[:, lo:512], v_sb[c][:],
                                         pt[:, 512:512 + N],
                                         start=(c == 0), stop=(c == nkc - 1))

                for c in range(nkc):
                    t = c - 4 * j
                    lo = 128 * t if t > 0 else 0
                    N = 512 - lo
                    # head A right-aligned in bank 0: the valid region
                    # [off:512+N] is contiguous -> ONE exp, no junk cols
                    off = 512 - N
                    qs = slice(jb + lo, jb + 512)
                    cs = slice(128 * c, 128 * c + 128)
                    sc2 = ps.tile([128, 1024], F32, tag="sc2", bufs=2,
                                  name=f"sc{j}_{c}_{m}")
                    nc.tensor.matmul(sc2[:, off:512], k2[0:64, cs],
                                     qr[m][0:64, qs], start=True, stop=True)
                    nc.tensor.matmul(sc2[:, 512:512 + N], k2[64:128, cs],
                                     qr[m][64:128, qs],
                                     start=True, stop=True)
                    pt = sb.tile([128, 1024], BF16, tag="probs", bufs=6,
                                 name=f"pr{j}_{c}_{m}")
                    nc.scalar.activation(pt[:, off:512 + N],
                                         sc2[:, off:512 + N],
                                         AF.Exp, bias=0.0, scale=0.125)
                    pop_filler()
                    flush_pv()
                    if c % 2 == 1:
                        pop_filler()
                    pend.append((c, t, off, N, pt))
                flush_pv()
                norm_stage(j, 2 * m, pv0)
                norm_stage(j, 2 * m + 1, pv1)

            def emit_h6_wave(j, nkc):
                """Head 6: chunks c0/c1 paired on the two array halves via
                qd6 = [q6; q6]; otherwise like a pair wave."""
                jb = 512 * j
                pv0 = ps.tile([128, 512], F32, tag="pv0", bufs=1,
                              name=f"pv{j}_6")
                pend = []

                def flush_pv():
                    while pend:
                        c0, c1, t0, t1, lo0, N0, lo1, N1, pt = pend.pop(0)
                        off0 = 512 - N0
                        if t0 >= 0:
                            nc.vector.tensor_tensor(
                                pt[:, off0:off0 + 128],
                                pt[:, off0:off0 + 128], tri01[:], ALU.mult)
                        if t1 >= 0:
                            nc.vector.tensor_tensor(
                                pt[:, 512:640], pt[:, 512:640], tri01[:],
                                ALU.mult)
                        nc.tensor.matmul(pv0[:, lo0:512], v_sb[c0][:],
                                         pt[:, off0:512],
                                         start=(c0 == 0), stop=False)
                        nc.tensor.matmul(pv0[:, lo1:512], v_sb[c1][:],
                                         pt[:, 512:512 + N1],
                                         start=False, stop=(c1 == nkc - 1))

                for p in range(nkc // 2):
                    c0, c1 = 2 * p, 2 * p + 1
                    t0, t1 = c0 - 4 * j, c1 - 4 * j
                    lo0 = 128 * t0 if t0 > 0 else 0
                    lo1 = 128 * t1 if t1 > 0 else 0
                    N0, N1 = 512 - lo0, 512 - lo1
                    off0 = 512 - N0
                    sc2 = ps.tile([128, 1024], F32, tag="sc2", bufs=2,
                                  name=f"sc{j}_h6_{p}")
                    nc.tensor.matmul(sc2[:, off0:512],
                                     k2[0:64, 128 * c0:128 * c0 + 128],
                                     qd6[0:64, jb + lo0:jb + 512],
                                     start=True, stop=True)
                    nc.tensor.matmul(sc2[:, 512:512 + N1],
                                     k2[64:128, 128 * c1:128 * c1 + 128],
                                     qd6[64:128, jb + lo1:jb + 512],
                                     start=True, stop=True)
                    pt = sb.tile([128, 1024], BF16, tag="probs", bufs=6,
                                 name=f"pr{j}_h6_{p}")
                    nc.scalar.activation(pt[:, off0:512 + N1],
                                         sc2[:, off0:512 + N1],
                                         AF.Exp, bias=0.0, scale=0.125)
                    pop_filler()
                    flush_pv()
                    pop_filler()
                    if len(filler_q) <= filler_reserve[0]:
                        keepalive()
                    pend.append((c0, c1, t0, t1, lo0, N0, lo1, N1, pt))
                flush_pv()
                norm_stage(j, 6, pv0)

            # ---- prologue: blocks 0+1 projection, rope(0) ----------------
            for m in (3, 4, 0, 1, 2):
                for _ in emit_qkv_chain(0, m):
                    pass
            for m in (3, 4, 0, 1, 2):
                for _ in emit_qkv_chain(1, m):
                    pass
            emit_rope(0, via_pe=True)
            emit_vtrans(0)
            dma_x(2)
            dma_x(3)

            # ---- main pipelined loop -------------------------------------
            # invariant entering iteration j: QKV blocks <= j+1 emitted,
            # rope/vtrans for blocks <= j done. Fillers inside attention(j):
            # QKV(j+2) first (deadline: rope(j+2)), then o_proj(j-1)/(j-2).
            qkv_gens = {}
            for j in range(NJ):
                nkc = 4 * j + 4
                # finalize block j-1 normalization here: it overlaps
                # attention(j) instead of serializing the wave boundary
                norm_fin()
                # QKV(j+2) first (deadline: rope(j+2) mid-next-block), then
                # o_proj: (j-2) tail units before (j-1)
                if j + 2 < NJ:
                    gens = [emit_qkv_chain(j + 2, m) for m in range(5)]
                    filler_q.extend(gens)
                    qkv_gens[j + 2] = gens
                if j >= 2:
                    filler_q.append(emit_oproj_pair(j - 2, 4))
                    filler_q.append(emit_oproj_unit(j - 2, 6))
                if j >= 1:
                    filler_q.append(emit_oproj_pair(j - 1, 0))
                    filler_q.append(emit_oproj_pair(j - 1, 2))
                    if j == NJ - 1:
                        filler_q.append(emit_oproj_pair(j - 1, 4))
                        filler_q.append(emit_oproj_unit(j - 1, 6))
                # at the last block, hold back fillers so the end-of-body
                # drain keeps the PE (and its HAM clock) busy into the tail
                filler_reserve[0] = 1 if j == NJ - 1 else 0

                for wv in range(4):
                    if wv == 2 and j + 1 < NJ:
                        # rope(j+1) needs QKV(j+1) complete — targeted drain
                        # (usually a no-op: pops have long finished it)
                        drain_gens(qkv_gens.pop(j + 1, ()))
                        emit_rope(j + 1, via_pe=(j + 1 <= 1))
                        emit_vtrans(j + 1)
                    if wv < 3:
                        emit_pair_wave(j, wv, nkc)
                    else:
                        emit_h6_wave(j, nkc)
                    if j == NJ - 1:
                        # last block: fin per wave so the tail's o_proj(3)
                        # isn't gated on a 7-head fin chain
                        norm_fin()

            # ---- tail: o_proj of the last block --------------------------
            # fin chains run on DVE/gpsimd while the reserved fillers and
            # the deferred o_proj(2) tail units keep the PE busy
            norm_fin()
            drain_fillers()
            for ot0 in (0, 2, 4):
                for _ in emit_oproj_pair(3, ot0, tail=True):
                    pass
                keepalive()
            for _ in emit_oproj_unit(3, 6, tail=True):
                pass

            if DEBUG:
                for m in range(5):
                    nc.sync.dma_start(dbg["dqkv"][128 * m:128 * m + 128, :],
                                      qkv[m][:])
                for m in range(4):
                    nc.sync.dma_start(dbg["dqr"][128 * m:128 * m + 128, :],
                                      qr[m][:])
                nc.sync.dma_start(dbg["dk2"][:], k2[:])
                for i in range(16):
                    nc.sync.dma_start(dbg["dv"][128 * i:128 * i + 128, :],
                                      v_sb[i][:])
                for i in range(4):
                    nc.sync.dma_start(dbg["dattn"][128 * i:128 * i + 128, :],
                                      attn_all[i][:])

    nc.compile()
    return nc


def _host_prep(inputs):
    import ml_dtypes
    bf16 = ml_dtypes.bfloat16
    hid = np.ascontiguousarray(np.asarray(inputs["hidden_states"], np.float32))
    pos = np.asarray(inputs["position_ids"])[0].astype(np.float32)
    Wq = np.asarray(inputs["Wq"], np.float32)
    bq = np.asarray(inputs["bq"], np.float32)
    Wk = np.asarray(inputs["Wk"], np.float32)
    bk = np.asarray(inputs["bk"], np.float32)
    Wv = np.asarray(inputs["Wv"], np.float32)
    bv = np.asarray(inputs["bv"], np.float32)
    Wo = np.asarray(inputs["Wo"], np.float32)

    inv = (1.0 / (ROPE_THETA ** (np.arange(0, HD, 2, dtype=np.float32) / HD))
           ).astype(np.float32)
    freqs = pos[:, None] * inv[None, :]
    emb = np.concatenate([freqs, freqs], -1)            # [S, 64]
    cosT = np.cos(emb).T.astype(np.float32)             # [64, S]
    sinT = np.sin(emb).T.astype(np.float32)
    sinm = sinT.copy()
    sinm[0:32] *= -1.0                                  # fold rotate_half sign
    cos2 = np.ascontiguousarray(np.vstack([cosT, cosT])).astype(bf16)
    sinm2 = np.ascontiguousarray(np.vstack([sinm, sinm])).astype(bf16)

    maps = []
    for b in range(B):
        for g in range(2):
            xT = np.ascontiguousarray(hid[b].T).astype(bf16)
            Wsl = np.concatenate([Wq[448 * g:448 * g + 448],
                                  Wk[64 * g:64 * g + 64],
                                  Wv[64 * g:64 * g + 64]], 0)
            wT = np.ascontiguousarray(Wsl.T).astype(bf16)   # [896, 576]
            bias = np.zeros(640, np.float32)
            bias[:576] = np.concatenate([bq[448 * g:448 * g + 448],
                                         bk[64 * g:64 * g + 64],
                                         bv[64 * g:64 * g + 64]])
            woT = np.ascontiguousarray(
                Wo[:, 448 * g:448 * g + 448].T).astype(bf16)
            pswap = np.zeros((128, 128), np.float32)
            for m in range(128):
                half, r = (m // 64) * 64, m % 64
                pswap[half + (r + 32) % 64, m] = 1.0
            dup2 = np.zeros((128, 128), np.float32)
            for p in range(128):
                for m in (p % 64, p % 64 + 64):
                    dup2[p, m] = 1.0
            maps.append(dict(xT=xT, wT=wT, bias=bias, woT=woT,
                             cos2=cos2, sinm2=sinm2,
                             ident64=np.eye(64, dtype=bf16),
                             pswap=pswap.astype(bf16),
                             dup2=dup2.astype(bf16)))
    return maps


def kernel(**inputs) -> np.ndarray:
    from concourse.bass_utils import run_bass_kernel_spmd

    if "nc" not in _PROGRAM_CACHE:
        _PROGRAM_CACHE["nc"] = _build_program()
    nc = _PROGRAM_CACHE["nc"]

    in_maps = _host_prep(inputs)
    res = run_bass_kernel_spmd(nc, in_maps, core_ids=list(range(8)),
                               **_PROGRAM_CACHE.get("run_kwargs", {}))
    _PROGRAM_CACHE["last_result"] = res
    yTs = [np.asarray(res.results[i]["yT"], np.float32) for i in range(8)]
    out = np.stack([(yTs[2 * b] + yTs[2 * b + 1]).T for b in range(B)], 0)
    return np.ascontiguousarray(out)
